# revision 10
# baseline (speedup 1.0000x reference)
"""Causal self-attention (B=4, T=2048, C=1024, H=16) on 8 trn2 NeuronCores.

Sharding: tensor-parallel over heads x data-parallel over batch.
Core c handles batch b=c//2 and head group g=c%2 (8 heads each).
Each core computes qkv projection for its heads, causal attention, and a
partial output projection; the host sums the two partial yT per batch and
adds the output bias.

Device dataflow is feature-major ("transposed") end to end:
  qkT[f, t]   = Wqk.T @ xT          (f = head-pair-blocked q/k features)
  scoresT[k, q] = kT.T @ qT         per head, k-tile=128 x q-tile=512
  e = exp(scoresT/8), causal-masked via affine_select
  avT[d(+1), q] += [v|1].T @ e      ones-column gives softmax denominator
  aoT = avT[0:64] * (1/avT[64]) broadcast (PE outer-product broadcast)
  yT_partial = Wo.T @ aoT
No transposes are needed anywhere; the host transposes x and y (free).
Heads are packed two per 128-partition block (even head at partitions 0-63,
odd at 64-127) so the K=64 score matmuls of a pair run row-tiled
concurrently in the PE array.

Mixed-precision fp8 fast path: QKV (chunks 1-3) and AV (q-tiles 1-3) run as
float8e4 DoubleRow matmuls (two 128-deep contraction tiles per instruction
at 2x rate).  Early tokens attend to few keys, so their attention is peaked
and fp8 quantization of e/v would pass straight through to the largest
outputs -- chunk 0 of QKV and q-tile 0 of attention therefore stay bf16.
Scores q/k (bf16) and the output projection (bf16) are full precision
everywhere.  exp is restricted to the causal window of each diagonal k-tile
(the affine_select zero-fills the rest of the tile).
"""

import os
import threading
from contextlib import ExitStack

import ml_dtypes
import numpy as np

import concourse.bass as bass
from concourse import bacc
import concourse.mybir as mybir
import concourse.tile as tile
from concourse.bass_utils import run_bass_kernel_spmd

B, T, C = 4, 2048, 1024
H, D = 16, 64
NCORES = 8
HL = 8                 # heads per core
NPAIR = HL // 2        # head pairs per core
CQK = 2 * HL * D       # 1024 local q+k features
CV = HL * D            # 512 local v features
TQ = 512               # query tile (PSUM bank limit for f32)
NQT = T // TQ          # 4
TK = 128               # key tile (PSUM partition limit)
NKT = T // TK          # 16
KO = C // 128          # 8 contraction tiles over C
F32 = mybir.dt.float32
BF16 = mybir.dt.bfloat16
FP8 = mybir.dt.float8e4
DR = mybir.MatmulPerfMode.DoubleRow
VW = D + 2             # v_aug width: 64 d + ones + pad (dual-fp8 ldweights
                       # needs even strides/counts)

FP8_QKV = os.environ.get("ATTN_FP8_QKV", "1") == "1"
FP8_AV = os.environ.get("ATTN_FP8_AV", "1") == "1"
CWIN = os.environ.get("ATTN_CWIN", "1") == "1"
YBF16 = os.environ.get("ATTN_YBF16", "1") == "1"
PBCAST = os.environ.get("ATTN_PBCAST", "0") == "1"

Y_DT = BF16 if YBF16 else F32

# float32r: full-precision fp32 data, fast PE streaming mode (1 cycle/row at
# N>=256 vs 4 for plain float32).
MM_DT = {
    "f32r": mybir.dt.float32r,
    "f32": mybir.dt.float32,
}[os.environ.get("ATTN_MM_DT", "f32r")]


def r(ap):
    """View an fp32 AP as the matmul input dtype (float32r needs producers to
    write through an fp32r-typed AP so the BIR verifier sees rounded data)."""
    if MM_DT == F32 or ap.dtype != F32:
        return ap
    return ap.bitcast(MM_DT)


def _mm(nc, out, lhsT, rhs, start=True, stop=True):
    nc.tensor.matmul(out, r(lhsT), r(rhs), start=start, stop=stop)


def _mmdr(nc, out, lhsT, rhs, start=True, stop=True):
    """fp8 DoubleRow matmul: lhsT [K,2,M], rhs [K,2,N] -> out [M,N] summed
    over the two packed k-tiles."""
    nc.tensor.matmul(out, lhsT, rhs, start=start, stop=stop, perf_mode=DR)


def build_program():
    nc = bacc.Bacc(None)
    # chunk-0 x in bf16 (early-token accuracy); the rest of x in fp8
    x0 = nc.declare_dram_parameter("x0", [C, TQ], BF16, isOutput=False)
    xT8 = nc.declare_dram_parameter("xT8", [C, T - TQ],
                                    FP8 if FP8_QKV else BF16, isOutput=False)
    wqk16 = nc.declare_dram_parameter("wqk16", [C, CQK], BF16, isOutput=False)
    wqk8 = nc.declare_dram_parameter("wqk8", [C, CQK], FP8, isOutput=False)
    bqk = nc.declare_dram_parameter("bqk", [CQK], F32, isOutput=False)
    wv16 = nc.declare_dram_parameter("wv16", [C, CV], BF16, isOutput=False)
    wv8 = nc.declare_dram_parameter("wv8", [C, CV], FP8, isOutput=False)
    bv = nc.declare_dram_parameter("bv", [CV], F32, isOutput=False)
    wo = nc.declare_dram_parameter("wo", [CV, C], BF16, isOutput=False)
    yT = nc.declare_dram_parameter("yT", [C, T], Y_DT, isOutput=True)

    with ExitStack() as ctx:
        ctx.enter_context(nc.allow_low_precision(reason="fp8/fp32r matmul inputs"))
        tc = ctx.enter_context(tile.TileContext(nc))
        persist = ctx.enter_context(tc.tile_pool(name="persist", bufs=1))
        p2 = ctx.enter_context(tc.tile_pool(name="p2", bufs=3))
        pw = ctx.enter_context(tc.tile_pool(name="pw", bufs=1))
        px = ctx.enter_context(tc.tile_pool(name="px", bufs=2))
        ps = ctx.enter_context(tc.tile_pool(name="ps", bufs=2, space="PSUM"))
        ps_av = ctx.enter_context(tc.tile_pool(name="ps_av", bufs=2, space="PSUM"))
        dram = ctx.enter_context(tc.tile_pool(name="dram", bufs=2, space="DRAM"))

        # q/k features, head-pair blocked: block m<4 = q of pair m
        # (even head partitions 0-63, odd 64-127), block 4+m = k of pair m.
        # One tile per 512-token chunk so chunk writes and attention reads
        # of different chunks never false-serialize (deps are per-tile).
        qkTs = [persist.tile([128, 8, TQ], BF16, name=f"qkT{c}")
                for c in range(NQT)]
        # v with ones column for the softmax denominator: [tok, kt, head, d+]
        # fp8 copies (width 66, even strides for dual-fp8 ldweights) feed the
        # DoubleRow AV of q-tiles 1-3; the bf16 chunk-0 copy feeds q-tile 0.
        v_augs8 = [persist.tile([128, TQ // TK, HL, VW], FP8,
                                name=f"vaug8_{c}") for c in range(NQT)]
        v_aug16s = [persist.tile([128, TQ // TK, HL, D + 1], BF16,
                                 name=f"vaug16_{c}") for c in range(NQT)]
        bqk_sb = persist.tile([128, 8], F32)
        bv_row = persist.tile([1, CV], F32)
        bvb_sb = persist.tile([128, CV], F32)    # v bias broadcast over tokens
        ones_sb = persist.tile([128, 128], F32)
        wo_sb = persist.tile([128, 4, C], BF16)
        # normalized attention output, one tile per head pair (per-tile deps:
        # the projection's per-ko reads then only wait on that pair's norm)
        aoTs = [persist.tile([128, T], BF16, name=f"aoT{p}")
                for p in range(NPAIR)]

        ones_f32 = persist.tile([128, 128], F32)
        nc.vector.memset(ones_f32, 1.0)
        nc.vector.tensor_copy(out=r(ones_sb[:]), in_=ones_f32)
        for c in range(NQT):
            nc.vector.tensor_copy(
                out=v_augs8[c][:, :, :, D:VW],
                in_=ones_f32[:, 0 : (TQ // TK) * HL * (VW - D)].rearrange(
                    "p (a b c) -> p a b c", a=TQ // TK, b=HL))
        for c in range(NQT):
            nc.vector.tensor_copy(
                out=v_aug16s[c][:, :, :, D : D + 1],
                in_=ones_f32[:, 0 : (TQ // TK) * HL].rearrange(
                    "p (a b c) -> p a b c", a=TQ // TK, b=HL))
        nc.sync.dma_start(out=bqk_sb, in_=bqk[:].rearrange("(m p) -> p m", p=128))
        nc.sync.dma_start(out=r(bv_row[:]), in_=r(bv[:].unsqueeze(0)))

        x0_r = x0[:].rearrange("(ko p) t -> p ko t", p=128)
        xT8_r = xT8[:].rearrange("(ko p) t -> p ko t", p=128)
        wv16_r = wv16[:].rearrange("(ko p) f -> p ko f", p=128)
        wqk16_r = wqk16[:].rearrange("(ko p) f -> p ko f", p=128)
        # chunk-0 x and the v weights load first (ko-halves for finer deps)
        # so the first v matmuls start as early as possible.
        KH = KO // 2
        xt0 = [px.tile([128, KH, TQ], BF16, name=f"xt0_{h}", tag=f"xt{h}")
               for h in range(2)]
        wv_sb = [pw.tile([128, KH, CV], BF16, name=f"wv_{h}", tag=f"wv{h}")
                 for h in range(2)]
        # two parallel DMA queues for the startup loads; h=0 halves first so
        # the first (half-contraction) v matmuls start after ~2MB, not 8MB
        for h in range(2):
            nc.sync.dma_start(out=r(xt0[h][:]),
                              in_=r(x0_r[:, h * KH : (h + 1) * KH, :]))
            nc.gpsimd.dma_start(out=r(wv_sb[h][:]),
                                in_=r(wv16_r[:, h * KH : (h + 1) * KH, :]))
        # pair-0 q/k blocks (m=0 and m=4) in their own tile so the first
        # scores only wait on 0.5MB of wqk, not 2MB
        wqk_a = pw.tile([128, KO, 2, TK], BF16)
        nc.sync.dma_start(out=r(wqk_a[:, :, 0, :]), in_=r(wqk16_r[:, :, 0:TK]))
        nc.sync.dma_start(out=r(wqk_a[:, :, 1, :]),
                          in_=r(wqk16_r[:, :, 4 * TK : 5 * TK]))
        wqk_rest = pw.tile([128, KO, 6, TK], BF16)
        nc.sync.dma_start(out=r(wqk_rest[:, :, 0:3, :]),
                          in_=r(wqk16_r[:, :, TK : 4 * TK].rearrange(
                              "p k (m f) -> p k m f", f=TK)))
        nc.sync.dma_start(out=r(wqk_rest[:, :, 3:6, :]),
                          in_=r(wqk16_r[:, :, 5 * TK : 8 * TK].rearrange(
                              "p k (m f) -> p k m f", f=TK)))

        def wqk16_block(ko, m):
            if m in (0, 4):
                return wqk_a[:, ko, m // 4, :]
            return wqk_rest[:, ko, m - 1 if m < 4 else m - 2, :]
        # fp8 weights for the chunk 1-3 QKV fast path
        wv8_sb = pw.tile([128, KO, CV], FP8)
        wqk8_sb = pw.tile([128, KO, CQK], FP8)
        if FP8_QKV:
            nc.gpsimd.dma_start(
                out=wv8_sb, in_=wv8[:].rearrange("(ko p) f -> p ko f", p=128))
            nc.gpsimd.dma_start(
                out=wqk8_sb, in_=wqk8[:].rearrange("(ko p) f -> p ko f", p=128))

        # v-bias broadcast over the 128 token partitions via K=1 outer product
        bvb_ps = ps.tile([128, CV], F32, tag="s")
        _mm(nc, bvb_ps, ones_sb[0:1, :], bv_row)
        nc.vector.tensor_copy(out=bvb_sb, in_=bvb_ps)

        def qkv_chunk_items_bf16(ch, xt, split_v=False):
            """Chunk-0 QKV in bf16, one closure per matmul group.  Also
            mirrors v into the fp8 copy for the q-tile 1-3 AV."""

            def v_mt(mt, kos=range(KO), acc_in=None):
                def f():
                    acc = acc_in or ps.tile([128, CV], F32, tag="s")
                    for ko in kos:
                        _mm(nc, acc,
                            xt[ko // KH][:, ko % KH, mt * TK : (mt + 1) * TK],
                            wv_sb[ko // KH][:, ko % KH, :],
                            start=ko == 0, stop=ko == KO - 1)
                    if kos[-1] == KO - 1:
                        nc.vector.tensor_add(
                            out=v_aug16s[ch][:, mt, :, 0:D],
                            in0=acc.rearrange("p (h d) -> p h d", d=D),
                            in1=bvb_sb.rearrange("p (h d) -> p h d", d=D))
                        if FP8_AV:
                            nc.vector.tensor_copy(
                                out=v_augs8[ch][:, mt, :, 0:D],
                                in_=v_aug16s[ch][:, mt, :, 0:D])
                    return acc
                return f

            def qk_m(m):
                def f():
                    acc = ps.tile([128, TQ], F32, tag="s")
                    for ko in range(KO):
                        _mm(nc, acc, wqk16_block(ko, m),
                            xt[ko // KH][:, ko % KH, :],
                            start=ko == 0, stop=ko == KO - 1)
                    nc.vector.tensor_scalar_add(
                        out=qkTs[ch][:, m, :], in0=acc,
                        scalar1=bqk_sb[:, m : m + 1])
                return f

            if split_v:
                # half-contraction interleave: the A halves need only the h=0
                # loads, so compute starts while h=1 is still in flight;
                # A/B pairs share two PSUM slots (A0 A1 B0 A2 B1 A3 B2 B3)
                accs = {}
                seq = [("a", 0), ("a", 1), ("b", 0), ("a", 2), ("b", 1),
                       ("a", 3), ("b", 2), ("b", 3)]
                def mk(kind, mt):
                    if kind == "a":
                        return lambda: accs.__setitem__(
                            mt, v_mt(mt, kos=list(range(KH)))())
                    return lambda: v_mt(mt, kos=list(range(KH, KO)),
                                        acc_in=accs[mt])()
                return [mk(k, m) for k, m in seq] + [qk_m(m) for m in range(8)]
            return [v_mt(mt) for mt in range(TQ // TK)] + \
                   [qk_m(m) for m in range(8)]

        def qkv_chunk_items_fp8(ch, xt):
            """Chunk 1-3 QKV as fp8 DoubleRow over ko pairs."""

            def v_mt(mt):
                def f():
                    acc = ps.tile([128, CV], F32, tag="s")
                    for p in range(4):
                        _mmdr(nc, acc,
                              xt[:, 2 * p : 2 * p + 2, mt * TK : (mt + 1) * TK],
                              wv8_sb[:, 2 * p : 2 * p + 2, :],
                              start=p == 0, stop=p == 3)
                    nc.vector.tensor_add(
                        out=v_augs8[ch][:, mt, :, 0:D],
                        in0=acc.rearrange("p (h d) -> p h d", d=D),
                        in1=bvb_sb.rearrange("p (h d) -> p h d", d=D))
                return f

            def qk_m(m):
                def f():
                    acc = ps.tile([128, TQ], F32, tag="s")
                    for p in range(4):
                        _mmdr(nc, acc,
                              wqk8_sb[:, 2 * p : 2 * p + 2,
                                      m * 128 : (m + 1) * 128],
                              xt[:, 2 * p : 2 * p + 2, :],
                              start=p == 0, stop=p == 3)
                    nc.vector.tensor_scalar_add(
                        out=qkTs[ch][:, m, :], in0=acc,
                        scalar1=bqk_sb[:, m : m + 1])
                return f

            return [v_mt(mt) for mt in range(TQ // TK)] + \
                   [qk_m(m) for m in range(8)]

        def load_chunk(ch):
            t0 = ch * TQ - TQ  # xT8 starts at token TQ
            if FP8_QKV:
                xt = px.tile([128, KO, TQ], FP8, name=f"xt_{ch}", tag="xt8")
                nc.sync.dma_start(out=xt, in_=xT8_r[:, :, t0 : t0 + TQ])
                return xt
            xt = [px.tile([128, KH, TQ], FP8, name=f"xt_{ch}_{h}",
                          tag=f"xt8{h}") for h in range(2)]
            for h in range(2):
                nc.sync.dma_start(
                    out=xt[h][:],
                    in_=xT8_r[:, h * KH : (h + 1) * KH, t0 : t0 + TQ])
            return xt

        # chunk 0 prologue: v and the first pair's q/k blocks run dense;
        # the other six q/k blocks spread into the attention stream (pair p
        # of qt0 only needs blocks p and 4+p, which land pairs ahead).
        items0 = qkv_chunk_items_bf16(0, xt0, split_v=True)
        vseq = items0[:8]
        order = [vseq[0], vseq[1], vseq[2], items0[8 + 0], vseq[3],
                 vseq[4], vseq[5], vseq[6], items0[8 + 4], vseq[7]]
        for f in order:
            f()
        rest0 = [items0[8 + m] for m in (1, 5, 2, 6, 3, 7)]
        # out-proj weights are not needed until much later; load them now so
        # the DMA does not compete with the startup x/wv/wqk loads.
        nc.sync.dma_start(out=wo_sb, in_=wo[:].rearrange("(ko p) f -> p ko f", p=128))

        def make_norm(pair, q0, av_E, av_O, pe_bcast=False):
            def norm():
                if pe_bcast:
                    # tail variant: broadcast denominators with a K=1 PE outer
                    # product (PE is idle here) instead of the DRAM bounce,
                    # skipping two DMA-completion latencies.
                    stage = p2.tile([128, 2 * TQ], F32, tag="rec", bufs=2)
                    nc.vector.tensor_copy(out=r(stage[64:65, 0:TQ]),
                                          in_=av_E[D : D + 1, :])
                    nc.vector.tensor_copy(out=r(stage[64:65, TQ : 2 * TQ]),
                                          in_=av_O[D : D + 1, :])
                    bc_ps = ps.tile([64, 2 * TQ], F32, tag="s")
                    _mm(nc, bc_ps[:, 0:TQ], ones_sb[64:65, 0:64],
                        stage[64:65, 0:TQ])
                    _mm(nc, bc_ps[:, TQ : 2 * TQ], ones_sb[64:65, 0:64],
                        stage[64:65, TQ : 2 * TQ])
                    bc_sb = p2.tile([64, 2 * TQ], F32, tag="recbc", bufs=2)
                    nc.vector.reciprocal_approx_fast(out=bc_sb, in_=bc_ps)
                    nc.vector.tensor_mul(
                        out=aoTs[pair][0:64, q0 : q0 + TQ],
                        in0=av_E[0:D, :], in1=bc_sb[:, 0:TQ])
                    ao_tmp = p2.tile([64, TQ], BF16, tag="aotmp")
                    nc.vector.tensor_mul(out=ao_tmp, in0=av_O[0:D, :],
                                         in1=bc_sb[:, TQ : 2 * TQ])
                    nc.sync.dma_start(out=aoTs[pair][64:128, q0 : q0 + TQ],
                                      in_=ao_tmp)
                    return
                # denominators (av row D) -> SBUF, then an on-chip
                # gpsimd partition-broadcast fans them over the 64
                # d-partitions (no DRAM bounce); the reciprocal then runs
                # 64-lane-parallel at partition 0 (reciprocal_approx_fast
                # misbehaves at base partition 64).
                stage = p2.tile([128, 2 * TQ], F32, tag="rec", bufs=2)
                nc.vector.tensor_copy(out=stage[64:65, 0:TQ],
                                      in_=av_E[D : D + 1, :])
                nc.vector.tensor_copy(out=stage[64:65, TQ : 2 * TQ],
                                      in_=av_O[D : D + 1, :])
                den_bc = p2.tile([64, 2 * TQ], F32, tag="bc", bufs=2)
                if PBCAST:
                    nc.gpsimd.partition_broadcast(den_bc[:], stage[64:65, :],
                                                  channels=64)
                else:
                    dr = dram.tile([1, 2 * TQ], F32, tag="drrec")
                    nc.sync.dma_start(out=dr, in_=stage[64:65, :])
                    nc.sync.dma_start(out=den_bc,
                                      in_=dr[:].to_broadcast([64, 2 * TQ]))
                bc_sb = p2.tile([64, 2 * TQ], F32, tag="recbc", bufs=2)
                nc.vector.reciprocal_approx_fast(out=bc_sb, in_=den_bc)
                nc.vector.tensor_mul(
                    out=aoTs[pair][0:64, q0 : q0 + TQ],
                    in0=av_E[0:D, :], in1=bc_sb[:, 0:TQ])
                ao_tmp = p2.tile([64, TQ], BF16, tag="aotmp")
                nc.vector.tensor_mul(out=ao_tmp, in0=av_O[0:D, :],
                                     in1=bc_sb[:, TQ : 2 * TQ])
                # odd head lives at partitions 64-127: DMA does the hop
                nc.sync.dma_start(out=aoTs[pair][64:128, q0 : q0 + TQ],
                                  in_=ao_tmp)
            return norm

        def make_proj(q0, ko_order=(0, 1, 2, 3)):
            def proj_m(m):
                def f():
                    acc = ps.tile([128, TQ], F32, tag="s")
                    for i, ko in enumerate(ko_order):
                        _mm(nc, acc, wo_sb[:, ko, m * 128 : (m + 1) * 128],
                            aoTs[ko][:, q0 : q0 + TQ], start=i == 0, stop=i == 3)
                    y_sb = p2.tile([128, TQ], Y_DT, tag="ysb", bufs=2)
                    nc.vector.tensor_copy(out=y_sb, in_=acc)
                    nc.sync.dma_start(
                        out=yT[m * 128 : (m + 1) * 128, q0 : q0 + TQ], in_=y_sb)
                return f
            return [proj_m(m) for m in range(8)]

        # Pending PE work spread one item per kt into the ACT-paced attention
        # stream: next chunk's QKV groups (deadline: before the next q-tile)
        # and the previous q-tile's projection (needs this qt's norms done).
        q_chunk = list(rest0)
        q_proj = []

        def pop_work(ktg):
            if q_chunk:
                q_chunk.pop(0)()
                if len(q_chunk) > 12:  # backlog: the queue must drain
                    q_chunk.pop(0)()   # before the next qt needs it
            elif q_proj and ktg >= 8:
                q_proj.pop(0)()

        for qt in range(NQT):
            q0 = qt * TQ
            nkt = (q0 + TQ) // TK  # causal: only k-tiles with k0 <= q0+TQ-1
            if qt + 1 < NQT:
                ch = qt + 1
                xt = load_chunk(ch)
                if FP8_QKV:
                    q_chunk.extend(qkv_chunk_items_fp8(ch, xt))
                else:
                    q_chunk.extend(qkv_chunk_items_bf16(ch, xt))
            ktg = 0
            pair_order = (1, 2, 3, 0) if qt == NQT - 1 else range(NPAIR)
            use_fp8_av = FP8_AV and qt > 0
            for pair in pair_order:
                av_E = ps_av.tile([VW, TQ], F32, tag="avE")
                av_O = ps_av.tile([VW, TQ], F32, tag="avO")

                if use_fp8_av:
                    # AV in fp8 DoubleRow: one matmul per head covers a
                    # k-tile PAIR (256 keys).  e tiles hold both kts of a
                    # pair: [128 k, 2 kt, 2 head, TQ q].
                    npr = nkt // 2

                    def av_pair(e_pair, i, w0):
                        # w0: known-zero cols below w0 are skipped (their e
                        # was never written); pair 0 always covers [0, TQ)
                        vc, vk = (2 * i) // (TQ // TK), (2 * i) % (TQ // TK)
                        _mmdr(nc, av_E[:, w0:TQ],
                              v_augs8[vc][:, vk : vk + 2, 2 * pair, :],
                              e_pair[:, 0, :, w0:TQ],
                              start=i == 0, stop=i == npr - 1)
                        _mmdr(nc, av_O[:, w0:TQ],
                              v_augs8[vc][:, vk : vk + 2, 2 * pair + 1, :],
                              e_pair[:, 1, :, w0:TQ],
                              start=i == 0, stop=i == npr - 1)

                    prev = None  # av deferred one pair so scores(i+1) sit
                    # ahead of av(i) in the PE queue: the PE computes scores
                    # while ACT exps the previous block.
                    for i in range(npr):
                        # head-major: [128 k, 2 head, 2 kt, TQ q] so the
                        # DoubleRow AV rhs [128, 2, TQ] is kt-contiguous
                        e_pair = p2.tile([128, 2, 2, TQ], FP8, tag="e")
                        c0p = max(0, 2 * i * TK - q0) if CWIN else 0
                        for j in range(2):
                            kt = 2 * i + j
                            k0 = kt * TK
                            c0 = max(0, k0 - q0) if CWIN else 0
                            kc, kk = k0 // TQ, k0 % TQ
                            s_ps = ps.tile([128, 2, TQ], F32, tag="s")
                            _mm(nc, s_ps[:, 0, c0:TQ],
                                qkTs[kc][0:64, 4 + pair, kk : kk + TK],
                                qkTs[qt][0:64, pair, c0:TQ])
                            _mm(nc, s_ps[:, 1, c0:TQ],
                                qkTs[kc][64:128, 4 + pair, kk : kk + TK],
                                qkTs[qt][64:128, pair, c0:TQ])
                            # e = exp(scores / sqrt(d_k)); no max-subtraction:
                            # scores/8 is O(1) here, exp cannot overflow.
                            nc.scalar.activation(
                                out=e_pair[:, :, j, c0:TQ],
                                in_=s_ps[:, :, c0:TQ],
                                func=mybir.ActivationFunctionType.Exp,
                                scale=0.125)
                            if k0 + TK - 1 > q0:
                                # diagonal: j=0 only needs the 128-wide
                                # triangle band; j=1 also zero-fills the
                                # all-masked [c0p, c0) that exp skipped
                                b0 = c0 if j == 0 else c0p
                                b1 = min(c0 + TK, TQ) if j == 0 else TQ
                                nc.gpsimd.affine_select(
                                    out=e_pair[:, :, j, b0:b1],
                                    in_=e_pair[:, :, j, b0:b1],
                                    compare_op=mybir.AluOpType.is_ge,
                                    fill=0.0, base=q0 - k0 + b0,
                                    pattern=[[0, 2], [1, b1 - b0]],
                                    channel_multiplier=-1)
                            ktg += 1
                            pop_work(ktg)
                        if prev is not None:
                            av_pair(*prev)
                        prev = (e_pair, i, c0p)
                    av_pair(*prev)
                else:
                    def av_mms(e_sb, kt):
                        vc, vk = kt // (TQ // TK), kt % (TQ // TK)
                        _mm(nc, av_E[0 : D + 1, :],
                            v_aug16s[vc][:, vk, 2 * pair, :],
                            e_sb[:, 0, :], start=kt == 0, stop=kt == nkt - 1)
                        _mm(nc, av_O[0 : D + 1, :],
                            v_aug16s[vc][:, vk, 2 * pair + 1, :],
                            e_sb[:, 1, :], start=kt == 0, stop=kt == nkt - 1)

                    prev = None
                    for kt in range(nkt):
                        k0 = kt * TK
                        c0 = max(0, k0 - q0) if CWIN else 0
                        kc, kk = k0 // TQ, k0 % TQ
                        s_ps = ps.tile([128, 2, TQ], F32, tag="s")
                        _mm(nc, s_ps[:, 0, c0:TQ],
                            qkTs[kc][0:64, 4 + pair, kk : kk + TK],
                            qkTs[qt][0:64, pair, c0:TQ])
                        _mm(nc, s_ps[:, 1, c0:TQ],
                            qkTs[kc][64:128, 4 + pair, kk : kk + TK],
                            qkTs[qt][64:128, pair, c0:TQ])
                        e_sb = p2.tile([128, 2, TQ], BF16, tag="e")
                        nc.scalar.activation(
                            out=e_sb[:, :, c0:TQ], in_=s_ps[:, :, c0:TQ],
                            func=mybir.ActivationFunctionType.Exp, scale=0.125)
                        if k0 + TK - 1 > q0:
                            nc.gpsimd.affine_select(
                                out=e_sb, in_=e_sb,
                                compare_op=mybir.AluOpType.is_ge,
                                fill=0.0, base=q0 - k0,
                                pattern=[[0, 2], [1, TQ]],
                                channel_multiplier=-1)
                        if prev is not None:
                            av_mms(*prev)
                        prev = (e_sb, kt)
                        ktg += 1
                        pop_work(ktg)
                    av_mms(*prev)
                make_norm(pair, q0, av_E, av_O,
                          pe_bcast=pair == list(pair_order)[-1])()
            q_proj.extend(make_proj(
                q0, ko_order=(1, 2, 3, 0) if qt == NQT - 1 else (0, 1, 2, 3)))
        for f in q_chunk:
            f()
        for f in q_proj:
            f()
    nc.finalize()
    return nc


_CACHE = threading.local()


def _get_program():
    nc = getattr(_CACHE, "nc", None)
    if nc is None:
        nc = build_program()
        _CACHE.nc = nc
    return nc


def _make_in_maps(x, W_qkv, b_qkv, W_out, b_out):
    x = np.asarray(x, np.float32)
    W_qkv = np.asarray(W_qkv, np.float32)
    b_qkv = np.asarray(b_qkv, np.float32)
    W_out = np.asarray(W_out, np.float32)
    in_maps = []
    bf16 = ml_dtypes.bfloat16
    fp8 = ml_dtypes.float8_e4m3
    for c in range(NCORES):
        b, g = c // 2, c % 2
        sl = slice(512 * g, 512 * g + 512)  # this head group's q (and k,v) cols
        xt = x[b].T
        wqk = np.concatenate(
            [W_qkv[:, 0:1024][:, sl], W_qkv[:, 1024:2048][:, sl]], axis=1)
        wv = W_qkv[:, 2048:3072][:, sl]
        in_maps.append({
            "x0": np.ascontiguousarray(xt[:, 0:TQ].astype(bf16)),
            "xT8": np.ascontiguousarray(
                xt[:, TQ:].astype(fp8 if FP8_QKV else bf16)),
            "wqk16": np.ascontiguousarray(wqk.astype(bf16)),
            "wqk8": np.ascontiguousarray(wqk.astype(fp8)),
            "bqk": np.ascontiguousarray(
                np.concatenate([b_qkv[0:1024][sl], b_qkv[1024:2048][sl]])),
            "wv16": np.ascontiguousarray(wv.astype(bf16)),
            "wv8": np.ascontiguousarray(wv.astype(fp8)),
            "bv": np.ascontiguousarray(b_qkv[2048:3072][sl]),
            "wo": np.ascontiguousarray(W_out[sl, :].astype(bf16)),
        })
    return in_maps


def _run(inputs, trace=False):
    nc = _get_program()
    in_maps = _make_in_maps(**inputs)
    res = run_bass_kernel_spmd(nc, in_maps, list(range(NCORES)), trace=trace)
    b_out = np.asarray(inputs["b_out"], np.float32)
    y = np.empty((B, T, C), np.float32)
    for b in range(B):
        yt = (res.results[2 * b]["yT"].astype(np.float32)
              + res.results[2 * b + 1]["yT"].astype(np.float32))
        y[b] = yt.T + b_out
    return y, res


def kernel(x, W_qkv, b_qkv, W_out, b_out):
    y, _ = _run(dict(x=x, W_qkv=W_qkv, b_qkv=b_qkv, W_out=W_out, b_out=b_out))
    return y


# revision 12
# speedup vs baseline: 1.0057x; 1.0057x over previous
"""Causal self-attention (B=4, T=2048, C=1024, H=16) on 8 trn2 NeuronCores.

Sharding: tensor-parallel over heads x data-parallel over batch.
Core c handles batch b=c//2 and head group g=c%2 (8 heads each).
Each core computes qkv projection for its heads, causal attention, and a
partial output projection; the host sums the two partial yT per batch and
adds the output bias.

Device dataflow is feature-major ("transposed") end to end:
  qkT[f, t]   = Wqk.T @ xT          (f = head-pair-blocked q/k features)
  scoresT[k, q] = kT.T @ qT         per head, k-tile=128 x q-tile=512
  e = exp(scoresT/8), causal-masked via affine_select
  avT[d(+1), q] += [v|1].T @ e      ones-column gives softmax denominator
  aoT = avT[0:64] * (1/avT[64]) broadcast (PE outer-product broadcast)
  yT_partial = Wo.T @ aoT
No transposes are needed anywhere; the host transposes x and y (free).
Heads are packed two per 128-partition block (even head at partitions 0-63,
odd at 64-127) so the K=64 score matmuls of a pair run row-tiled
concurrently in the PE array.

Mixed-precision fp8 fast path: QKV (chunks 1-3) and AV (q-tiles 1-3) run as
float8e4 DoubleRow matmuls (two 128-deep contraction tiles per instruction
at 2x rate).  Early tokens attend to few keys, so their attention is peaked
and fp8 quantization of e/v would pass straight through to the largest
outputs -- chunk 0 of QKV and q-tile 0 of attention therefore stay bf16.
Scores q/k (bf16) and the output projection (bf16) are full precision
everywhere.  exp is restricted to the causal window of each diagonal k-tile
(the affine_select zero-fills the rest of the tile).
"""

import os
import threading
from contextlib import ExitStack

import ml_dtypes
import numpy as np

import concourse.bass as bass
from concourse import bacc
import concourse.mybir as mybir
import concourse.tile as tile
from concourse.bass_utils import run_bass_kernel_spmd

B, T, C = 4, 2048, 1024
H, D = 16, 64
NCORES = 8
HL = 8                 # heads per core
NPAIR = HL // 2        # head pairs per core
CQK = 2 * HL * D       # 1024 local q+k features
CV = HL * D            # 512 local v features
TQ = 512               # query tile (PSUM bank limit for f32)
NQT = T // TQ          # 4
TK = 128               # key tile (PSUM partition limit)
NKT = T // TK          # 16
KO = C // 128          # 8 contraction tiles over C
F32 = mybir.dt.float32
BF16 = mybir.dt.bfloat16
FP8 = mybir.dt.float8e4
DR = mybir.MatmulPerfMode.DoubleRow
VW = D + 2             # v_aug width: 64 d + ones + pad (dual-fp8 ldweights
                       # needs even strides/counts)

FP8_QKV = os.environ.get("ATTN_FP8_QKV", "1") == "1"
FP8_AV = os.environ.get("ATTN_FP8_AV", "1") == "1"
CWIN = os.environ.get("ATTN_CWIN", "1") == "1"
YBF16 = os.environ.get("ATTN_YBF16", "1") == "1"
PBCAST = os.environ.get("ATTN_PBCAST", "0") == "1"

Y_DT = BF16 if YBF16 else F32

# float32r: full-precision fp32 data, fast PE streaming mode (1 cycle/row at
# N>=256 vs 4 for plain float32).
MM_DT = {
    "f32r": mybir.dt.float32r,
    "f32": mybir.dt.float32,
}[os.environ.get("ATTN_MM_DT", "f32r")]


def r(ap):
    """View an fp32 AP as the matmul input dtype (float32r needs producers to
    write through an fp32r-typed AP so the BIR verifier sees rounded data)."""
    if MM_DT == F32 or ap.dtype != F32:
        return ap
    return ap.bitcast(MM_DT)


def _mm(nc, out, lhsT, rhs, start=True, stop=True):
    nc.tensor.matmul(out, r(lhsT), r(rhs), start=start, stop=stop)


def _mmdr(nc, out, lhsT, rhs, start=True, stop=True):
    """fp8 DoubleRow matmul: lhsT [K,2,M], rhs [K,2,N] -> out [M,N] summed
    over the two packed k-tiles."""
    nc.tensor.matmul(out, lhsT, rhs, start=start, stop=stop, perf_mode=DR)


def build_program():
    nc = bacc.Bacc(None)
    # chunk-0 x in bf16 (early-token accuracy); the rest of x in fp8
    x0 = nc.declare_dram_parameter("x0", [C, TQ], BF16, isOutput=False)
    xT8 = nc.declare_dram_parameter("xT8", [C, T - TQ],
                                    FP8 if FP8_QKV else BF16, isOutput=False)
    wqk16 = nc.declare_dram_parameter("wqk16", [C, CQK], BF16, isOutput=False)
    wqk8 = nc.declare_dram_parameter("wqk8", [C, CQK], FP8, isOutput=False)
    bqk = nc.declare_dram_parameter("bqk", [CQK], F32, isOutput=False)
    wv16 = nc.declare_dram_parameter("wv16", [C, CV], BF16, isOutput=False)
    wv8 = nc.declare_dram_parameter("wv8", [C, CV], FP8, isOutput=False)
    bv = nc.declare_dram_parameter("bv", [CV], F32, isOutput=False)
    wo = nc.declare_dram_parameter("wo", [CV, C], BF16, isOutput=False)
    yT = nc.declare_dram_parameter("yT", [C, T], Y_DT, isOutput=True)

    with ExitStack() as ctx:
        ctx.enter_context(nc.allow_low_precision(reason="fp8/fp32r matmul inputs"))
        tc = ctx.enter_context(tile.TileContext(nc))
        persist = ctx.enter_context(tc.tile_pool(name="persist", bufs=1))
        p2 = ctx.enter_context(tc.tile_pool(name="p2", bufs=3))
        pw = ctx.enter_context(tc.tile_pool(name="pw", bufs=1))
        px = ctx.enter_context(tc.tile_pool(name="px", bufs=2))
        ps = ctx.enter_context(tc.tile_pool(name="ps", bufs=3, space="PSUM"))
        ps_av = ctx.enter_context(tc.tile_pool(name="ps_av", bufs=1, space="PSUM"))
        dram = ctx.enter_context(tc.tile_pool(name="dram", bufs=2, space="DRAM"))

        # q/k features, head-pair blocked: block m<4 = q of pair m
        # (even head partitions 0-63, odd 64-127), block 4+m = k of pair m.
        # One tile per 512-token chunk so chunk writes and attention reads
        # of different chunks never false-serialize (deps are per-tile).
        qkTs = [persist.tile([128, 8, TQ], BF16, name=f"qkT{c}")
                for c in range(NQT)]
        # v with ones column for the softmax denominator: [tok, kt, head, d+]
        # fp8 copies (width 66, even strides for dual-fp8 ldweights) feed the
        # DoubleRow AV of q-tiles 1-3; the bf16 chunk-0 copy feeds q-tile 0.
        v_augs8 = [persist.tile([128, TQ // TK, HL, VW], FP8,
                                name=f"vaug8_{c}") for c in range(NQT)]
        v_aug16s = [persist.tile([128, TQ // TK, HL, D + 1], BF16,
                                 name=f"vaug16_{c}") for c in range(NQT)]
        bqk_sb = persist.tile([128, 8], F32)
        bv_row = persist.tile([1, CV], F32)
        bvb_sb = persist.tile([128, CV], F32)    # v bias broadcast over tokens
        ones_sb = persist.tile([128, 128], F32)
        wo_sb = persist.tile([128, 4, C], BF16)
        # normalized attention output, one tile per head pair (per-tile deps:
        # the projection's per-ko reads then only wait on that pair's norm)
        aoTs = [persist.tile([128, T], BF16, name=f"aoT{p}")
                for p in range(NPAIR)]

        ones_f32 = persist.tile([128, 128], F32)
        nc.vector.memset(ones_f32, 1.0)
        nc.vector.tensor_copy(out=r(ones_sb[:]), in_=ones_f32)
        for c in range(NQT):
            nc.vector.tensor_copy(
                out=v_augs8[c][:, :, :, D:VW],
                in_=ones_f32[:, 0 : (TQ // TK) * HL * (VW - D)].rearrange(
                    "p (a b c) -> p a b c", a=TQ // TK, b=HL))
        for c in range(NQT):
            nc.vector.tensor_copy(
                out=v_aug16s[c][:, :, :, D : D + 1],
                in_=ones_f32[:, 0 : (TQ // TK) * HL].rearrange(
                    "p (a b c) -> p a b c", a=TQ // TK, b=HL))
        nc.scalar.dma_start(out=bqk_sb,
                            in_=bqk[:].rearrange("(m p) -> p m", p=128))
        nc.scalar.dma_start(out=r(bv_row[:]), in_=r(bv[:].unsqueeze(0)))

        x0_r = x0[:].rearrange("(ko p) t -> p ko t", p=128)
        xT8_r = xT8[:].rearrange("(ko p) t -> p ko t", p=128)
        wv16_r = wv16[:].rearrange("(ko p) f -> p ko f", p=128)
        wqk16_r = wqk16[:].rearrange("(ko p) f -> p ko f", p=128)
        # chunk-0 x and the v weights load first (ko-halves for finer deps)
        # so the first v matmuls start as early as possible.
        KH = KO // 2
        xt0 = [px.tile([128, KH, TQ], BF16, name=f"xt0_{h}", tag=f"xt{h}")
               for h in range(2)]
        wv_sb = [pw.tile([128, KH, CV], BF16, name=f"wv_{h}", tag=f"wv{h}")
                 for h in range(2)]
        # two parallel DMA queues for the startup loads; h=0 halves first so
        # the first (half-contraction) v matmuls start after ~2MB, not 8MB
        # startup loads fan out over four DMA queues so the prologue is
        # fed after ~3us instead of serializing ~2MB on one queue
        nc.sync.dma_start(out=r(xt0[0][:]), in_=r(x0_r[:, 0:KH, :]))
        nc.scalar.dma_start(out=r(xt0[1][:]), in_=r(x0_r[:, KH:KO, :]))
        for h in range(2):
            nc.gpsimd.dma_start(out=r(wv_sb[h][:]),
                                in_=r(wv16_r[:, h * KH : (h + 1) * KH, :]))
        # pair-0 q/k blocks (m=0 and m=4) in their own tile so the first
        # scores only wait on 0.5MB of wqk, not 2MB
        wqk_a = pw.tile([128, KO, 2, TK], BF16)
        nc.scalar.dma_start(out=r(wqk_a[:, :, 0, :]),
                            in_=r(wqk16_r[:, :, 0:TK]))
        nc.scalar.dma_start(out=r(wqk_a[:, :, 1, :]),
                            in_=r(wqk16_r[:, :, 4 * TK : 5 * TK]))
        wqk_rest = pw.tile([128, KO, 6, TK], BF16)
        nc.sync.dma_start(out=r(wqk_rest[:, :, 0:3, :]),
                          in_=r(wqk16_r[:, :, TK : 4 * TK].rearrange(
                              "p k (m f) -> p k m f", f=TK)))
        nc.sync.dma_start(out=r(wqk_rest[:, :, 3:6, :]),
                          in_=r(wqk16_r[:, :, 5 * TK : 8 * TK].rearrange(
                              "p k (m f) -> p k m f", f=TK)))

        def wqk16_block(ko, m):
            if m in (0, 4):
                return wqk_a[:, ko, m // 4, :]
            return wqk_rest[:, ko, m - 1 if m < 4 else m - 2, :]
        # fp8 weights for the chunk 1-3 QKV fast path
        wv8_sb = pw.tile([128, KO, CV], FP8)
        wqk8_sb = pw.tile([128, KO, CQK], FP8)
        if FP8_QKV:
            nc.gpsimd.dma_start(
                out=wv8_sb, in_=wv8[:].rearrange("(ko p) f -> p ko f", p=128))
            nc.gpsimd.dma_start(
                out=wqk8_sb, in_=wqk8[:].rearrange("(ko p) f -> p ko f", p=128))

        # v-bias broadcast over the 128 token partitions via K=1 outer product
        bvb_ps = ps.tile([128, CV], F32, tag="s")
        _mm(nc, bvb_ps, ones_sb[0:1, :], bv_row)
        nc.vector.tensor_copy(out=bvb_sb, in_=bvb_ps)

        def qkv_chunk_items_bf16(ch, xt, split_v=False):
            """Chunk-0 QKV in bf16, one closure per matmul group.  Also
            mirrors v into the fp8 copy for the q-tile 1-3 AV."""

            def v_mt(mt, kos=range(KO), acc_in=None):
                def f():
                    acc = acc_in or ps.tile([128, CV], F32, tag="s")
                    for ko in kos:
                        _mm(nc, acc,
                            xt[ko // KH][:, ko % KH, mt * TK : (mt + 1) * TK],
                            wv_sb[ko // KH][:, ko % KH, :],
                            start=ko == 0, stop=ko == KO - 1)
                    if kos[-1] == KO - 1:
                        nc.vector.tensor_add(
                            out=v_aug16s[ch][:, mt, :, 0:D],
                            in0=acc.rearrange("p (h d) -> p h d", d=D),
                            in1=bvb_sb.rearrange("p (h d) -> p h d", d=D))
                        if FP8_AV:
                            nc.vector.tensor_copy(
                                out=v_augs8[ch][:, mt, :, 0:D],
                                in_=v_aug16s[ch][:, mt, :, 0:D])
                    return acc
                return f

            def qk_m(m):
                def f():
                    acc = ps.tile([128, TQ], F32, tag="s")
                    for ko in range(KO):
                        _mm(nc, acc, wqk16_block(ko, m),
                            xt[ko // KH][:, ko % KH, :],
                            start=ko == 0, stop=ko == KO - 1)
                    nc.vector.tensor_scalar_add(
                        out=qkTs[ch][:, m, :], in0=acc,
                        scalar1=bqk_sb[:, m : m + 1])
                return f

            if split_v:
                # half-contraction interleave: the A halves need only the h=0
                # loads, so compute starts while h=1 is still in flight;
                # A/B pairs share two PSUM slots (A0 A1 B0 A2 B1 A3 B2 B3)
                accs = {}
                seq = [("a", 0), ("a", 1), ("b", 0), ("a", 2), ("b", 1),
                       ("a", 3), ("b", 2), ("b", 3)]
                def mk(kind, mt):
                    if kind == "a":
                        return lambda: accs.__setitem__(
                            mt, v_mt(mt, kos=list(range(KH)))())
                    return lambda: v_mt(mt, kos=list(range(KH, KO)),
                                        acc_in=accs[mt])()
                return [mk(k, m) for k, m in seq] + [qk_m(m) for m in range(8)]
            return [v_mt(mt) for mt in range(TQ // TK)] + \
                   [qk_m(m) for m in range(8)]

        def qkv_chunk_items_fp8(ch, xt):
            """Chunk 1-3 QKV as fp8 DoubleRow over ko pairs."""

            def v_mt(mt):
                def f():
                    acc = ps.tile([128, CV], F32, tag="s")
                    for p in range(4):
                        _mmdr(nc, acc,
                              xt[:, 2 * p : 2 * p + 2, mt * TK : (mt + 1) * TK],
                              wv8_sb[:, 2 * p : 2 * p + 2, :],
                              start=p == 0, stop=p == 3)
                    nc.vector.tensor_add(
                        out=v_augs8[ch][:, mt, :, 0:D],
                        in0=acc.rearrange("p (h d) -> p h d", d=D),
                        in1=bvb_sb.rearrange("p (h d) -> p h d", d=D))
                return f

            def qk_m(m):
                def f():
                    acc = ps.tile([128, TQ], F32, tag="s")
                    for p in range(4):
                        _mmdr(nc, acc,
                              wqk8_sb[:, 2 * p : 2 * p + 2,
                                      m * 128 : (m + 1) * 128],
                              xt[:, 2 * p : 2 * p + 2, :],
                              start=p == 0, stop=p == 3)
                    nc.vector.tensor_scalar_add(
                        out=qkTs[ch][:, m, :], in0=acc,
                        scalar1=bqk_sb[:, m : m + 1])
                return f

            return [v_mt(mt) for mt in range(TQ // TK)] + \
                   [qk_m(m) for m in range(8)]

        def load_chunk(ch):
            t0 = ch * TQ - TQ  # xT8 starts at token TQ
            if FP8_QKV:
                xt = px.tile([128, KO, TQ], FP8, name=f"xt_{ch}", tag="xt8")
                nc.sync.dma_start(out=xt, in_=xT8_r[:, :, t0 : t0 + TQ])
                return xt
            xt = [px.tile([128, KH, TQ], FP8, name=f"xt_{ch}_{h}",
                          tag=f"xt8{h}") for h in range(2)]
            for h in range(2):
                nc.sync.dma_start(
                    out=xt[h][:],
                    in_=xT8_r[:, h * KH : (h + 1) * KH, t0 : t0 + TQ])
            return xt

        # chunk 0 prologue: v and the first pair's q/k blocks run dense;
        # the other six q/k blocks spread into the attention stream (pair p
        # of qt0 only needs blocks p and 4+p, which land pairs ahead).
        items0 = qkv_chunk_items_bf16(0, xt0, split_v=True)
        vseq = items0[:8]
        order = [vseq[0], vseq[1], vseq[2], items0[8 + 0], vseq[3],
                 vseq[4], vseq[5], vseq[6], items0[8 + 4], vseq[7]]
        for f in order:
            f()
        rest0 = [items0[8 + m] for m in (1, 5, 2, 6, 3, 7)]
        # out-proj weights are not needed until much later; load them now so
        # the DMA does not compete with the startup x/wv/wqk loads.
        nc.sync.dma_start(out=wo_sb, in_=wo[:].rearrange("(ko p) f -> p ko f", p=128))

        def make_norm(pair, q0, av_sb, pe_bcast=False):
            av_E = av_sb[:, 0, :]
            av_O = av_sb[:, 1, :]
            def norm():
                if pe_bcast:
                    # tail variant: broadcast denominators with a K=1 PE outer
                    # product (PE is idle here) instead of the DRAM bounce,
                    # skipping two DMA-completion latencies.
                    stage = p2.tile([128, 2 * TQ], F32, tag="rec", bufs=2)
                    nc.vector.tensor_copy(out=r(stage[64:65, 0:TQ]),
                                          in_=av_E[D : D + 1, :])
                    nc.vector.tensor_copy(out=r(stage[64:65, TQ : 2 * TQ]),
                                          in_=av_O[D : D + 1, :])
                    bc_ps = ps.tile([64, 2 * TQ], F32, tag="s")
                    _mm(nc, bc_ps[:, 0:TQ], ones_sb[64:65, 0:64],
                        stage[64:65, 0:TQ])
                    _mm(nc, bc_ps[:, TQ : 2 * TQ], ones_sb[64:65, 0:64],
                        stage[64:65, TQ : 2 * TQ])
                    bc_sb = p2.tile([64, 2 * TQ], F32, tag="recbc", bufs=2)
                    nc.vector.reciprocal_approx_fast(out=bc_sb, in_=bc_ps)
                    nc.vector.tensor_mul(
                        out=aoTs[pair][0:64, q0 : q0 + TQ],
                        in0=av_E[0:D, :], in1=bc_sb[:, 0:TQ])
                    ao_tmp = p2.tile([64, TQ], BF16, tag="aotmp")
                    nc.vector.tensor_mul(out=ao_tmp, in0=av_O[0:D, :],
                                         in1=bc_sb[:, TQ : 2 * TQ])
                    nc.sync.dma_start(out=aoTs[pair][64:128, q0 : q0 + TQ],
                                      in_=ao_tmp)
                    return
                # denominators (av row D) -> SBUF, then an on-chip
                # gpsimd partition-broadcast fans them over the 64
                # d-partitions (no DRAM bounce); the reciprocal then runs
                # 64-lane-parallel at partition 0 (reciprocal_approx_fast
                # misbehaves at base partition 64).
                stage = p2.tile([128, 2 * TQ], F32, tag="rec", bufs=2)
                nc.vector.tensor_copy(out=stage[64:65, 0:TQ],
                                      in_=av_E[D : D + 1, :])
                nc.vector.tensor_copy(out=stage[64:65, TQ : 2 * TQ],
                                      in_=av_O[D : D + 1, :])
                den_bc = p2.tile([64, 2 * TQ], F32, tag="bc", bufs=2)
                if PBCAST:
                    nc.gpsimd.partition_broadcast(den_bc[:], stage[64:65, :],
                                                  channels=64)
                else:
                    dr = dram.tile([1, 2 * TQ], F32, tag="drrec")
                    nc.sync.dma_start(out=dr, in_=stage[64:65, :])
                    nc.sync.dma_start(out=den_bc,
                                      in_=dr[:].to_broadcast([64, 2 * TQ]))
                bc_sb = p2.tile([64, 2 * TQ], F32, tag="recbc", bufs=2)
                nc.vector.reciprocal_approx_fast(out=bc_sb, in_=den_bc)
                nc.vector.tensor_mul(
                    out=aoTs[pair][0:64, q0 : q0 + TQ],
                    in0=av_E[0:D, :], in1=bc_sb[:, 0:TQ])
                ao_tmp = p2.tile([64, TQ], BF16, tag="aotmp")
                nc.vector.tensor_mul(out=ao_tmp, in0=av_O[0:D, :],
                                     in1=bc_sb[:, TQ : 2 * TQ])
                # odd head lives at partitions 64-127: DMA does the hop
                nc.sync.dma_start(out=aoTs[pair][64:128, q0 : q0 + TQ],
                                  in_=ao_tmp)
            return norm

        def make_proj(q0, ko_order=(0, 1, 2, 3)):
            def proj_m(m):
                def f():
                    acc = ps.tile([128, TQ], F32, tag="s")
                    for i, ko in enumerate(ko_order):
                        _mm(nc, acc, wo_sb[:, ko, m * 128 : (m + 1) * 128],
                            aoTs[ko][:, q0 : q0 + TQ], start=i == 0, stop=i == 3)
                    y_sb = p2.tile([128, TQ], Y_DT, tag="ysb", bufs=2)
                    nc.vector.tensor_copy(out=y_sb, in_=acc)
                    nc.sync.dma_start(
                        out=yT[m * 128 : (m + 1) * 128, q0 : q0 + TQ], in_=y_sb)
                return f
            return [proj_m(m) for m in range(8)]

        # Pending PE work spread one item per kt into the ACT-paced attention
        # stream: next chunk's QKV groups (deadline: before the next q-tile)
        # and the previous q-tile's projection (needs this qt's norms done).
        q_chunk = list(rest0)
        q_proj = []

        def pop_work(ktg):
            if q_chunk:
                q_chunk.pop(0)()
                if len(q_chunk) > 12:  # backlog: the queue must drain
                    q_chunk.pop(0)()   # before the next qt needs it
            elif q_proj and ktg >= 8:
                q_proj.pop(0)()

        for qt in range(NQT):
            q0 = qt * TQ
            nkt = (q0 + TQ) // TK  # causal: only k-tiles with k0 <= q0+TQ-1
            if qt + 1 < NQT:
                ch = qt + 1
                xt = load_chunk(ch)
                if FP8_QKV:
                    q_chunk.extend(qkv_chunk_items_fp8(ch, xt))
                else:
                    q_chunk.extend(qkv_chunk_items_bf16(ch, xt))
            ktg = 0
            pair_order = (1, 2, 3, 0) if qt == NQT - 1 else range(NPAIR)
            use_fp8_av = FP8_AV and qt > 0
            for pair in pair_order:
                av_E = ps_av.tile([VW, TQ], F32, tag="avE")
                av_O = ps_av.tile([VW, TQ], F32, tag="avO")

                if use_fp8_av:
                    # AV in fp8 DoubleRow: one matmul per head covers a
                    # k-tile PAIR (256 keys).  e tiles hold both kts of a
                    # pair: [128 k, 2 kt, 2 head, TQ q].
                    npr = nkt // 2

                    def av_pair(e_pair, i, w0):
                        # w0: known-zero cols below w0 are skipped (their e
                        # was never written); pair 0 always covers [0, TQ)
                        vc, vk = (2 * i) // (TQ // TK), (2 * i) % (TQ // TK)
                        _mmdr(nc, av_E[:, w0:TQ],
                              v_augs8[vc][:, vk : vk + 2, 2 * pair, :],
                              e_pair[:, 0, :, w0:TQ],
                              start=i == 0, stop=i == npr - 1)
                        _mmdr(nc, av_O[:, w0:TQ],
                              v_augs8[vc][:, vk : vk + 2, 2 * pair + 1, :],
                              e_pair[:, 1, :, w0:TQ],
                              start=i == 0, stop=i == npr - 1)

                    prev = None  # av deferred one pair so scores(i+1) sit
                    # ahead of av(i) in the PE queue: the PE computes scores
                    # while ACT exps the previous block.
                    for i in range(npr):
                        # head-major: [128 k, 2 head, 2 kt, TQ q] so the
                        # DoubleRow AV rhs [128, 2, TQ] is kt-contiguous
                        e_pair = p2.tile([128, 2, 2, TQ], FP8, tag="e")
                        c0p = max(0, 2 * i * TK - q0) if CWIN else 0
                        for j in range(2):
                            kt = 2 * i + j
                            k0 = kt * TK
                            c0 = max(0, k0 - q0) if CWIN else 0
                            kc, kk = k0 // TQ, k0 % TQ
                            s_ps = ps.tile([128, 2, TQ], F32, tag="s")
                            _mm(nc, s_ps[:, 0, c0:TQ],
                                qkTs[kc][0:64, 4 + pair, kk : kk + TK],
                                qkTs[qt][0:64, pair, c0:TQ])
                            _mm(nc, s_ps[:, 1, c0:TQ],
                                qkTs[kc][64:128, 4 + pair, kk : kk + TK],
                                qkTs[qt][64:128, pair, c0:TQ])
                            # e = exp(scores / sqrt(d_k)); no max-subtraction:
                            # scores/8 is O(1) here, exp cannot overflow.
                            nc.scalar.activation(
                                out=e_pair[:, :, j, c0:TQ],
                                in_=s_ps[:, :, c0:TQ],
                                func=mybir.ActivationFunctionType.Exp,
                                scale=0.125)
                            if k0 + TK - 1 > q0:
                                # diagonal: j=0 only needs the 128-wide
                                # triangle band; j=1 also zero-fills the
                                # all-masked [c0p, c0) that exp skipped
                                b0 = c0 if j == 0 else c0p
                                b1 = min(c0 + TK, TQ) if j == 0 else TQ
                                nc.gpsimd.affine_select(
                                    out=e_pair[:, :, j, b0:b1],
                                    in_=e_pair[:, :, j, b0:b1],
                                    compare_op=mybir.AluOpType.is_ge,
                                    fill=0.0, base=q0 - k0 + b0,
                                    pattern=[[0, 2], [1, b1 - b0]],
                                    channel_multiplier=-1)
                            ktg += 1
                            pop_work(ktg)
                        if prev is not None:
                            av_pair(*prev)
                        prev = (e_pair, i, c0p)
                        av_pair(*prev)
                    av_sb = p2.tile([VW, 2, TQ], F32, tag="avsb", bufs=2)
                    nc.vector.tensor_copy(out=av_sb[:, 0, :], in_=av_E)
                    nc.vector.tensor_copy(out=av_sb[:, 1, :], in_=av_O)
                else:
                    def av_mms(e_sb, kt):
                        vc, vk = kt // (TQ // TK), kt % (TQ // TK)
                        _mm(nc, av_E[0 : D + 1, :],
                            v_aug16s[vc][:, vk, 2 * pair, :],
                            e_sb[:, 0, :], start=kt == 0, stop=kt == nkt - 1)
                        _mm(nc, av_O[0 : D + 1, :],
                            v_aug16s[vc][:, vk, 2 * pair + 1, :],
                            e_sb[:, 1, :], start=kt == 0, stop=kt == nkt - 1)

                    prev = None
                    for kt in range(nkt):
                        k0 = kt * TK
                        c0 = max(0, k0 - q0) if CWIN else 0
                        kc, kk = k0 // TQ, k0 % TQ
                        s_ps = ps.tile([128, 2, TQ], F32, tag="s")
                        _mm(nc, s_ps[:, 0, c0:TQ],
                            qkTs[kc][0:64, 4 + pair, kk : kk + TK],
                            qkTs[qt][0:64, pair, c0:TQ])
                        _mm(nc, s_ps[:, 1, c0:TQ],
                            qkTs[kc][64:128, 4 + pair, kk : kk + TK],
                            qkTs[qt][64:128, pair, c0:TQ])
                        e_sb = p2.tile([128, 2, TQ], BF16, tag="e")
                        nc.scalar.activation(
                            out=e_sb[:, :, c0:TQ], in_=s_ps[:, :, c0:TQ],
                            func=mybir.ActivationFunctionType.Exp, scale=0.125)
                        if k0 + TK - 1 > q0:
                            nc.gpsimd.affine_select(
                                out=e_sb, in_=e_sb,
                                compare_op=mybir.AluOpType.is_ge,
                                fill=0.0, base=q0 - k0,
                                pattern=[[0, 2], [1, TQ]],
                                channel_multiplier=-1)
                        if prev is not None:
                            av_mms(*prev)
                        prev = (e_sb, kt)
                        ktg += 1
                        pop_work(ktg)
                    av_mms(*prev)
                    av_sb = p2.tile([VW, 2, TQ], F32, tag="avsb", bufs=2)
                    nc.vector.tensor_copy(out=av_sb[:, 0, :], in_=av_E)
                    nc.vector.tensor_copy(out=av_sb[:, 1, :], in_=av_O)
                make_norm(pair, q0, av_sb,
                          pe_bcast=pair == list(pair_order)[-1])()
            q_proj.extend(make_proj(
                q0, ko_order=(1, 2, 3, 0) if qt == NQT - 1 else (0, 1, 2, 3)))
        for f in q_chunk:
            f()
        for f in q_proj:
            f()
    nc.finalize()
    return nc


_CACHE = threading.local()


def _get_program():
    nc = getattr(_CACHE, "nc", None)
    if nc is None:
        nc = build_program()
        _CACHE.nc = nc
    return nc


def _make_in_maps(x, W_qkv, b_qkv, W_out, b_out):
    x = np.asarray(x, np.float32)
    W_qkv = np.asarray(W_qkv, np.float32)
    b_qkv = np.asarray(b_qkv, np.float32)
    W_out = np.asarray(W_out, np.float32)
    in_maps = []
    bf16 = ml_dtypes.bfloat16
    fp8 = ml_dtypes.float8_e4m3
    for c in range(NCORES):
        b, g = c // 2, c % 2
        sl = slice(512 * g, 512 * g + 512)  # this head group's q (and k,v) cols
        xt = x[b].T
        wqk = np.concatenate(
            [W_qkv[:, 0:1024][:, sl], W_qkv[:, 1024:2048][:, sl]], axis=1)
        wv = W_qkv[:, 2048:3072][:, sl]
        in_maps.append({
            "x0": np.ascontiguousarray(xt[:, 0:TQ].astype(bf16)),
            "xT8": np.ascontiguousarray(
                xt[:, TQ:].astype(fp8 if FP8_QKV else bf16)),
            "wqk16": np.ascontiguousarray(wqk.astype(bf16)),
            "wqk8": np.ascontiguousarray(wqk.astype(fp8)),
            "bqk": np.ascontiguousarray(
                np.concatenate([b_qkv[0:1024][sl], b_qkv[1024:2048][sl]])),
            "wv16": np.ascontiguousarray(wv.astype(bf16)),
            "wv8": np.ascontiguousarray(wv.astype(fp8)),
            "bv": np.ascontiguousarray(b_qkv[2048:3072][sl]),
            "wo": np.ascontiguousarray(W_out[sl, :].astype(bf16)),
        })
    return in_maps


def _run(inputs, trace=False):
    nc = _get_program()
    in_maps = _make_in_maps(**inputs)
    res = run_bass_kernel_spmd(nc, in_maps, list(range(NCORES)), trace=trace)
    b_out = np.asarray(inputs["b_out"], np.float32)
    y = np.empty((B, T, C), np.float32)
    for b in range(B):
        yt = (res.results[2 * b]["yT"].astype(np.float32)
              + res.results[2 * b + 1]["yT"].astype(np.float32))
        y[b] = yt.T + b_out
    return y, res


def kernel(x, W_qkv, b_qkv, W_out, b_out):
    y, _ = _run(dict(x=x, W_qkv=W_qkv, b_qkv=b_qkv, W_out=W_out, b_out=b_out))
    return y
